# revision 1
# baseline (speedup 1.0000x reference)
"""Trainium2 Bass kernel for the BasicQuadRGBV2 demosaic model.

Data-parallel over batch: 1 image per NeuronCore (8 cores).

Per-core dataflow (image [4,512,512] -> [3,1024,1024]):
  Phase 1  (conv stacks): two 3-layer CNNs (4->12->12->12, 3x3, relu) computed
           as block-banded matmuls. Layout: partitions = (y_row_window x chan),
           free dim = x. The y-taps of each 3x3 conv live inside a banded lhsT
           (contract over (y_in, c)); the x-taps are 3 PSUM-accumulated matmuls
           over free-dim-shifted views. Strips of 8 output rows; the output
           grid drifts +1 row per layer so PSUM evictions always land at
           natural partitions; strip-to-strip halo rows move via tiny DMAs.
  Phase 2  (softmax green): E=exp(relu(w3)), i=relu(f3); selector matmuls
           reduce over channels-in-partitions giving g0num/g1num/den planes.
  Phase 2.5: rden=1/den; g0,g1; chroma c1=mosaic1-g0, c2=mosaic2-g1.
  Phase 3  (chroma 5x5 convs): in pixel-shuffled space each needed
           (conv, phase) output is a 12-tap stencil over (c1,c2) within a
           3x3 quad-space window -> same banded-matmul machinery (y-band in
           partitions, 3 x-passes), 6 outputs at once.
  Phase 4  (assembly): DVE writes with stride-2 free APs interleave quad
           planes into full-res rows; contiguous row DMAs to DRAM.

All conv matmuls run as float32r (full PE rate at N=512).
"""

import numpy as np

import concourse.bass as bass
import concourse.tile as tile
from concourse import bacc, mybir
from concourse.tile import add_dep_helper as _adh


def add_dep(frm, to, reason=""):
    _adh(frm.ins, to.ins, reason=reason)
from concourse.bass_utils import run_bass_kernel_spmd

F32 = mybir.dt.float32
import os
F32R = mybir.dt.float32 if os.environ.get("K_FP32") else mybir.dt.float32r
RELU = mybir.ActivationFunctionType.Relu
EXP = mybir.ActivationFunctionType.Exp

WIDTH = 12
HW = 512  # image H = W
NSTRIP = 65  # strips s = -1 .. 63, stride 8


# ---------------------------------------------------------------- host prep

def _band_lhsT(W, cin):
    """W: [12, cin, 3, 3] -> [3, 10*cin, 96] banded matrices (one per x-tap).

    lhsT_dx[(yi*cin + c), (yo*12 + oc)] = W[oc, c, yi - yo, dx]
    """
    K, M = 10 * cin, 8 * WIDTH
    out = np.zeros((3, K, M), np.float32)
    for dx in range(3):
        for yo in range(8):
            for dy in range(3):
                yi = yo + dy
                out[dx, yi * cin:(yi + 1) * cin, yo * WIDTH:(yo + 1) * WIDTH] = \
                    W[:, :, dy, dx].T
    return out


def _selectors():
    selA = np.zeros((96, 24), np.float32)  # applied to i*E
    selB = np.zeros((96, 24), np.float32)  # applied to E
    for yl in range(8):
        for c in range(WIDTH):
            p = yl * WIDTH + c
            if c < 6:
                selA[p, yl * 3 + 0] = 1.0
            else:
                selA[p, yl * 3 + 1] = 1.0
            selB[p, yl * 3 + 2] = 1.0
    return selA, selB


def _g_stencil(K5, py, px):
    """12-tap quad-space stencil of a 5x5 conv output at phase (py,px),
    over chroma channels c1 (phase (0,1)) and c2 (phase (1,0))."""
    G = np.zeros((2, 3, 3), np.float32)
    for cc, (qy, qx) in enumerate(((0, 1), (1, 0))):
        for dy in (-1, 0, 1):
            for dx in (-1, 0, 1):
                d5y = 2 * dy + 2 - py + qy
                d5x = 2 * dx + 2 - px + qx
                if 0 <= d5y < 5 and 0 <= d5x < 5:
                    G[cc, dy + 1, dx + 1] = K5[d5y, d5x]
    return G


def _chroma_lhsT(chw, cvw, cqw):
    """-> [3, 36, 96] banded matrices for the 6 (conv, phase) outputs.

    Output order o: 0 ch@(0,0), 1 ch@(1,1), 2 cv@(0,0), 3 cv@(1,1),
                    4 cq@(1,0), 5 cq@(0,1).
    """
    specs = [(chw, 0, 0), (chw, 1, 1), (cvw, 0, 0), (cvw, 1, 1),
             (cqw, 1, 0), (cqw, 0, 1)]
    out = np.zeros((3, 64, 96), np.float32)
    for o, (K5, py, px) in enumerate(specs):
        G = _g_stencil(np.asarray(K5)[0, 0], py, px)
        for dx in range(3):
            for yo in range(16):
                for dy in (-1, 0, 1):
                    yi = yo + dy + 1
                    for cc in range(2):
                        out[dx, cc * 32 + yi, yo * 6 + o] = G[cc, dy + 1, dx]
    return out


def _host_prep(inputs):
    mosaic = np.asarray(inputs["mosaic"], np.float32)  # [8,4,512,512]
    mospad = np.zeros((mosaic.shape[0], 4, 522, 514), np.float32)
    mospad[:, :, 8:520, 1:513] = mosaic
    w1 = np.stack([_band_lhsT(np.asarray(inputs["fw0"]), 4),
                   _band_lhsT(np.asarray(inputs["ww0"]), 4)])  # [2,3,40,96]
    w23 = np.stack([_band_lhsT(np.asarray(inputs["fw1"]), 12),
                    _band_lhsT(np.asarray(inputs["ww1"]), 12),
                    _band_lhsT(np.asarray(inputs["fw2"]), 12),
                    _band_lhsT(np.asarray(inputs["ww2"]), 12)])  # [4,3,120,96]
    selA, selB = _selectors()
    sel = np.stack([selA, selB])  # [2,96,24]
    w5 = _chroma_lhsT(inputs["chw"], inputs["cvw"], inputs["cqw"])  # [3,64,96]
    return mospad, {"w1": w1, "w23": w23, "sel": sel, "w5": w5}


# ---------------------------------------------------------------- kernel IR

def _dma_rows_to_plane(nc, plane, src_ap, y_start, nrows, clip=HW):
    """DMA nrows of src (row r -> image row y_start+r) into a [128, 4*512]
    plane laid out y -> (partition y%128, free (y//128)*512 + x).
    Splits at 128-partition boundaries, clips y to [0, clip)."""
    y0, y1 = max(y_start, 0), min(y_start + nrows, clip)
    while y0 < y1:
        run = min(y1 - y0, 128 - (y0 % 128))
        r0 = y0 - y_start
        p0 = y0 % 128
        f0 = (y0 // 128) * 512
        nc.sync.dma_start(plane[p0:p0 + run, f0:f0 + 512],
                          src_ap[r0:r0 + run, :])
        y0 += run


def build_kernel(tc, outs, ins, ctx):
    nc = tc.nc
    mospad, w1, w23, sel, w5 = (ins[k] for k in
                                ("mospad", "w1", "w23", "sel", "w5"))
    out = outs["out"]

    wp = ctx.enter_context(tc.tile_pool(name="weights", bufs=1))
    pp = ctx.enter_context(tc.tile_pool(name="planes", bufs=1))
    ps = ctx.enter_context(tc.tile_pool(name="ps", bufs=6, space="PSUM"))
    ps2 = ctx.enter_context(tc.tile_pool(name="ps2", bufs=2, space="PSUM"))
    pools = {}
    for tag in ("b0", "b1f", "b1w", "b2f", "b2w", "b3"):
        pools[tag] = ctx.enter_context(tc.tile_pool(name=f"p_{tag}", bufs=4))
    ph2 = ctx.enter_context(tc.tile_pool(name="ph2", bufs=3))
    qpp = ctx.enter_context(tc.tile_pool(name="qp", bufs=1))
    asmp = ctx.enter_context(tc.tile_pool(name="asm", bufs=4))

    # --- weights to SBUF
    w1_t = wp.tile([40, 6 * 96], F32R, tag="w1")
    for st in range(2):
        for dx in range(3):
            nc.sync.dma_start(w1_t[:, (st * 3 + dx) * 96:(st * 3 + dx + 1) * 96],
                              w1[st, dx])
    w23_t = wp.tile([120, 12 * 96], F32R, tag="w23")
    for ly in range(4):
        for dx in range(3):
            nc.sync.dma_start(
                w23_t[:, (ly * 3 + dx) * 96:(ly * 3 + dx + 1) * 96], w23[ly, dx])
    sel_t = wp.tile([96, 48], F32R, tag="sel")
    nc.sync.dma_start(sel_t[:, 0:24], sel[0])
    nc.sync.dma_start(sel_t[:, 24:48], sel[1])
    w5_t = wp.tile([64, 3 * 96], F32R, tag="w5")
    for dx in range(3):
        nc.sync.dma_start(w5_t[:, dx * 96:(dx + 1) * 96], w5[dx])

    # --- persistent planes [128, 2048]: y -> (y%128, (y//128)*512 + x)
    mos_p = []
    for c in range(4):
        p = pp.tile([128, 2048], F32, tag=f"mos{c}")
        for t in range(4):
            nc.sync.dma_start(p[:, t * 512:(t + 1) * 512],
                              mospad[c, 8 + t * 128:8 + (t + 1) * 128,
                                     1:513].bitcast(F32))
        mos_p.append(p)
    g3 = pp.tile([128, 3 * 2048], F32, tag="g3")
    g0n = g3[:, 0:2048]
    g1n = g3[:, 2048:4096]
    den = g3[:, 4096:6144]
    c1p = pp.tile([128, 2048], F32R, tag="c1")
    c2p = pp.tile([128, 2048], F32R, tag="c2")
    zt = pp.tile([96, 514], F32R, tag="zt")
    nc.gpsimd.memset(zt[:].bitcast(F32), 0.0)

    # --- phase 1+2 wavefront over strips
    b0_t, b1_t, b2_t = {}, {}, {}  # s -> tile handles; b1/b2: (s, stack)

    def load_b0(s):
        t = pools["b0"].tile([40, 514], F32R, tag="b0", name="b0")
        src = mospad[:, 8 * s + 8:8 * s + 18, :].transpose([1, 0, 2])
        d = nc.sync.dma_start(t[:], src)
        b0_t[s] = (t, [d])

    def conv_pass(rhs_tile, kdim, w_tile, wofs, deps=()):
        psum = ps.tile([96, 512], F32, tag="cps", name="cps")

        def w(dx):
            return w_tile[0:kdim, wofs + dx * 96:wofs + (dx + 1) * 96]

        r = rhs_tile[0:kdim, :]
        mms = [nc.tensor.matmul(psum[:], w(dx), r[:, dx:dx + 512],
                                start=(dx == 0), stop=(dx == 2))
               for dx in range(3)]
        for mm in mms:
            for dep in deps:
                add_dep(mm, dep, reason="rhs-ready")
        return psum

    def evict_relu(psum, store, s, tag, k):
        # strip rows m=0..7 hold y = 8s+k+m; rows outside [0,512) must be
        # exactly zero (conv zero-padding) or they leak into the next layer
        t = pools[tag].tile([120, 514], F32R, tag=tag, name=tag)
        a = nc.scalar.activation(t[0:96, 1:513], psum[:], RELU)
        z0 = nc.gpsimd.memset(t[0:96, 0:514:513].bitcast(F32), 0.0)
        add_dep(z0, a, reason="pad-cols")
        insts = [a, z0]
        if s == -1 and 8 - k > 0:
            z = nc.sync.dma_start(t[0:(8 - k) * 12, :], zt[0:(8 - k) * 12, :])
            add_dep(z, a, reason="zero-pad-rows")
            add_dep(z, z0, reason="zero-pad-rows")
            insts.append(z)
        if s == 63 and 8 - k < 8:
            z = nc.sync.dma_start(t[(8 - k) * 12:96, :], zt[0:k * 12, :])
            add_dep(z, a, reason="zero-pad-rows")
            add_dep(z, z0, reason="zero-pad-rows")
            insts.append(z)
        store[(s, tag)] = (t, insts)

    def halo(store, s, tag, eng):
        # store[(s,tag)][96:120] <- store[(s+1,tag)][0:24]  (rows y+8, y+9)
        dst, insts = store[(s, tag)]
        if (s + 1, tag) in store:
            d = eng.dma_start(dst[96:120, :], store[(s + 1, tag)][0][0:24, :])
        else:
            d = eng.dma_start(dst[96:120, :], zt[0:24, :])
        for i_ in insts:
            add_dep(d, i_, reason="halo-after-evict")

    def phase2(s, psf, psw):
        it = ph2.tile([96, 512], F32R, tag="i")
        et = ph2.tile([96, 512], F32R, tag="e")
        nc.scalar.activation(it[:], psf[:], RELU)
        nc.scalar.activation(et[:], psw[:], EXP)
        nc.vector.tensor_scalar_max(et[:], et[:], 1.0)
        nc.vector.tensor_mul(it[:], it[:], et[:])  # i*E in place
        p2 = ps2.tile([24, 512], F32, tag="p2")
        nc.tensor.matmul(p2[:], sel_t[:, 0:24],
                         it[:], start=True, stop=False)
        nc.tensor.matmul(p2[:], sel_t[:, 24:48],
                         et[:], start=False, stop=True)
        s2 = ph2.tile([24, 512], F32, tag="s2")
        nc.vector.tensor_copy(s2[:], p2[:])
        ys = 8 * s + 3
        ya, yb = max(ys, 0), min(ys + 8, HW)
        while ya < yb:
            run = min(yb - ya, 128 - (ya % 128))
            p0 = ya % 128
            dst = g3[p0:p0 + run, :].rearrange(
                "p (s c x) -> p s c x", s=3, c=4)[:, :, ya // 128, :]
            sv = s2[(ya - ys) * 3:(ya - ys + run) * 3, :]
            nc.gpsimd.dma_start(dst, sv)
            ya += run

    # --- phases 2.5/3/4 as chunked functions, interleaved into the wavefront
    asm_specs = [  # (ch, py, px, qp index or None, plane addend or None)
        (0, 0, 0, 0, mos_p[0]), (0, 0, 1, None, mos_p[1]),
        (0, 1, 0, 4, g1n), (0, 1, 1, 3, mos_p[3]),
        (1, 0, 0, None, mos_p[0]), (1, 0, 1, None, g0n),
        (1, 1, 0, None, g1n), (1, 1, 1, None, mos_p[3]),
        (2, 0, 0, 2, mos_p[0]), (2, 0, 1, 5, g0n),
        (2, 1, 0, None, mos_p[2]), (2, 1, 1, 1, mos_p[3]),
    ]
    qp6_h = {}

    def phase25(t):
        # green + chroma for y rows 128t..128t+127 (free chunk t of planes)
        cs = slice(t * 512, (t + 1) * 512)
        nc.vector.reciprocal(den[:, cs], den[:, cs])
        nc.vector.tensor_mul(g0n[:, cs], g0n[:, cs], den[:, cs])  # g0
        nc.vector.tensor_mul(g1n[:, cs], g1n[:, cs], den[:, cs])  # g1
        nc.vector.tensor_sub(c1p[:, cs], mos_p[1][:, cs], g0n[:, cs])
        nc.vector.tensor_sub(c2p[:, cs], mos_p[2][:, cs], g1n[:, cs])

    def phase3_strip(sq):
        h = sq // 16
        if h not in qp6_h:
            qp6_h[h] = qpp.tile([128, 6 * 1024], F32, tag="qp6",
                                name=f"qp6_{h}")
        qp6 = qp6_h[h]
        b3 = pools["b3"].tile([64, 514], F32R, tag="b3", name="b3")
        wrs = [nc.gpsimd.memset(b3[0:64, 0:514:513].bitcast(F32), 0.0)]
        y0 = 16 * sq - 1
        if y0 < 0:
            wrs.append(nc.gpsimd.memset(b3[0:1, :].bitcast(F32), 0.0))
            wrs.append(nc.gpsimd.memset(b3[32:33, :].bitcast(F32), 0.0))
        if y0 + 18 > HW:
            wrs.append(nc.sync.dma_start(b3[17:18, :], zt[0:1, :]))
            wrs.append(nc.sync.dma_start(b3[49:50, :], zt[0:1, :]))
        for cc, src_plane in ((0, c1p), (1, c2p)):
            ya, yb = max(y0, 0), min(y0 + 18, HW)
            while ya < yb:
                run = min(yb - ya, 128 - (ya % 128))
                d = nc.sync.dma_start(
                    b3[cc * 32 + ya - y0:cc * 32 + ya - y0 + run, 1:513],
                    src_plane[ya % 128:ya % 128 + run,
                              (ya // 128) * 512:(ya // 128) * 512 + 512])
                wrs.append(d)
                ya += run
        wrs.append(nc.gpsimd.dma_start(b3[18:32, :], zt[0:14, :]))
        wrs.append(nc.gpsimd.dma_start(b3[50:64, :], zt[0:14, :]))
        p3 = ps.tile([96, 512], F32, tag="cps", name="p3")
        mm3 = [nc.tensor.matmul(p3[:], w5_t[:, dx * 96:(dx + 1) * 96],
                                b3[0:64, dx:dx + 512],
                                start=(dx == 0), stop=(dx == 2))
               for dx in range(3)]
        for mm in mm3:
            for wr in wrs:
                add_dep(mm, wr, reason="b3-ready")
        s3 = ph2.tile([96, 512], F32, tag="s3")
        nc.vector.tensor_copy(s3[:], p3[:])
        yq = 16 * sq
        tlc = (yq // 128) - 2 * h  # 0 or 1: 512-chunk within the half
        dst = qp6[yq % 128:yq % 128 + 16, :].rearrange(
            "p (o c x) -> p o c x", o=6, c=2)[:, :, tlc, :]
        nc.sync.dma_start(dst, s3[:])

    def assemble_half(h):
        qp6 = qp6_h[h]
        for tl in range(2):
            t = 2 * h + tl
            for ch in range(3):
                for py in range(2):
                    a = asmp.tile([128, 1024], F32, tag="asm", name="asm")
                    prev = None
                    for (c_, py_, px, qo, addend) in asm_specs:
                        if c_ != ch or py_ != py:
                            continue
                        view = a[:].rearrange("p (x two) -> p two x",
                                              two=2)[:, px, :]
                        if qo is None:
                            src = addend[:, t * 512:(t + 1) * 512]
                            w_ = nc.scalar.copy(view, src)
                        else:
                            w_ = nc.vector.tensor_add(
                                view,
                                qp6[:, qo * 1024 + tl * 512:
                                    qo * 1024 + tl * 512 + 512],
                                addend[:, t * 512:(t + 1) * 512])
                        if prev is not None:
                            add_dep(w_, prev, reason="asm-interleave")
                        prev = w_
                    dst = out[ch].rearrange("(y two) x -> two y x", two=2)[
                        py, t * 128:(t + 1) * 128, :]
                    nc.sync.dma_start(dst, a[:])

    # interleave: after phase2(t3) finishes the last strip of plane-chunk t
    # (t3 == 16t+15), emit that chunk's green/chroma and the phase-3 strips
    # it unlocks; assembly of each half follows its last phase-3 strip.
    def emit_chunk(t):
        phase25(t)
        lo = max(0, 8 * t - 1)
        hi = min(32, 8 * t + 7 + (1 if t == 3 else 0))
        for sq in range(lo, hi):
            phase3_strip(sq)
            if sq == 15:
                assemble_half(0)
            if sq == 31:
                assemble_half(1)

    for i in range(NSTRIP + 4):
        s = i - 1  # L1 strip index
        if s <= 63:
            load_b0(s)
            for st, tag in ((0, "b1f"), (1, "b1w")):
                evict_relu(conv_pass(b0_t[s][0], 40, w1_t, st * 3 * 96,
                                     deps=b0_t[s][1]), b1_t, s, tag, 1)
            if s - 1 >= -1:
                b0_t.pop(s - 1, None)
        t2 = s - 2  # L2 strip index (skewed: halo source already evicted)
        if -1 <= t2 <= 63:
            halo(b1_t, t2, "b1f", nc.gpsimd)
            halo(b1_t, t2, "b1w", nc.sync)
            for st, (tag_in, tag_out) in enumerate((("b1f", "b2f"),
                                                    ("b1w", "b2w"))):
                evict_relu(conv_pass(b1_t[(t2, tag_in)][0], 120, w23_t,
                                     st * 3 * 96), b2_t, t2, tag_out, 2)
        t3 = s - 4  # L3 strip index (skewed)
        if -1 <= t3 <= 63:
            for tag in ("b2f", "b2w"):
                halo(b2_t, t3, tag, nc.sync)
            psf = conv_pass(b2_t[(t3, "b2f")][0], 120, w23_t, 2 * 3 * 96)
            psw = conv_pass(b2_t[(t3, "b2w")][0], 120, w23_t, 3 * 3 * 96)
            phase2(t3, psf, psw)
            for tag in ("b1f", "b1w"):
                b1_t.pop((t3, tag), None)
            if t3 - 1 >= -1:
                for tag in ("b2f", "b2w"):
                    b2_t.pop((t3 - 1, tag), None)
            if t3 in (15, 31, 47, 63):
                emit_chunk(t3 // 16)



_CACHE = {}


def _get_compiled():
    if "nc" in _CACHE:
        return _CACHE["nc"]
    nc = bacc.Bacc("TRN2", target_bir_lowering=False, debug=False,
                   enable_asserts=False)
    ins = {
        "mospad": nc.dram_tensor("mospad", [4, 522, 514], F32R,
                                 kind="ExternalInput").ap(),
        "w1": nc.dram_tensor("w1", [2, 3, 40, 96], F32R,
                             kind="ExternalInput").ap(),
        "w23": nc.dram_tensor("w23", [4, 3, 120, 96], F32R,
                              kind="ExternalInput").ap(),
        "sel": nc.dram_tensor("sel", [2, 96, 24], F32R,
                              kind="ExternalInput").ap(),
        "w5": nc.dram_tensor("w5", [3, 64, 96], F32R,
                             kind="ExternalInput").ap(),
    }
    outs = {"out": nc.dram_tensor("out", [3, 1024, 1024], F32,
                                  kind="ExternalOutput").ap()}
    from contextlib import ExitStack
    with tile.TileContext(nc) as tc, ExitStack() as ctx:
        build_kernel(tc, outs, ins, ctx)
    nc.compile()
    _CACHE["nc"] = nc
    return nc


def kernel(**inputs):
    nc = _get_compiled()
    mospad, shared = _host_prep(inputs)
    in_maps = []
    for b in range(8):
        m = {"mospad": np.ascontiguousarray(mospad[b])}
        m.update(shared)
        in_maps.append(m)
    res = run_bass_kernel_spmd(nc, in_maps, core_ids=list(range(8)))
    return np.stack([res.results[b]["out"] for b in range(8)])



# revision 4
# speedup vs baseline: 1.1739x; 1.1739x over previous
"""Trainium2 Bass kernel for the BasicQuadRGBV2 demosaic model (v2).

Data-parallel over batch: 1 image per NeuronCore (8 cores).

Per-core dataflow (image [4,512,512] -> [3,1024,1024]), all-bf16 conv path:
  Phase 1: two 3-layer CNNs as banded matmuls (y-rows x chan in partitions,
    x in free, 3 PSUM-accumulated x-tap passes). The f and w stacks share a
    [96,1024] two-bank PSUM tile, so one relu-eviction (PSUM f32 -> bf16)
    serves both stacks; tiles are [120, 2x514] (f|w halves), halo rows via a
    single bf16 copy per layer from the next strip's tile.
  Phase 2: E=exp(psw) (ACT), et=max(E,1), i=relu(psf), m=i*et (bf16); two
    bf16 selector matmuls pack 4 strips into one [128,512] PSUM which is
    scattered by DMA straight into the g0n/g1n/den f32 planes (no staging).
  Phase 2.5: rden=1/den, g*=rden, bf16 chroma planes c12 = mos12 - g01.
  Phase 3: 5x5 chroma convs in quad space as bf16 banded matmuls ([36,514]
    gathered from c12 in one DMA); PSUM is DMA'd directly into the qp6
    quad-plane layout (no eviction op).
  Phase 4: interleave quad planes (DVE/ACT) per 128-row chunk, DMA to DRAM.
"""

import numpy as np
import ml_dtypes

import concourse.bass as bass
import concourse.tile as tile
from concourse import bacc, mybir
from concourse.tile import add_dep_helper as _adh
from concourse.bass_utils import run_bass_kernel_spmd


def add_dep(frm, to, reason=""):
    _adh(frm.ins, to.ins, reason=reason)


F32 = mybir.dt.float32
F32R = mybir.dt.float32r
BF16 = mybir.dt.bfloat16
RELU = mybir.ActivationFunctionType.Relu
EXP = mybir.ActivationFunctionType.Exp
BF = ml_dtypes.bfloat16

WIDTH = 12
HW = 512


# ---------------------------------------------------------------- host prep

def _band(W, cin, nin):
    """W [12, cin, 3, 3] -> [3, nin*cin, 96]: lhsT_dx[(yi*cin+c), (yo*12+oc)]
    = W[oc, c, yi-yo, dx] (bf16-rounded)."""
    Wq = np.asarray(W, np.float32)
    out = np.zeros((3, nin * cin, 96), np.float32)
    for dx in range(3):
        for yo in range(8):
            for dy in range(3):
                yi = yo + dy
                if yi < nin:
                    out[dx, yi * cin:(yi + 1) * cin,
                        yo * WIDTH:(yo + 1) * WIDTH] = Wq[:, :, dy, dx].T
    return out


def _g_stencil(K5, py, px):
    G = np.zeros((2, 3, 3), np.float32)
    for cc, (qy, qx) in enumerate(((0, 1), (1, 0))):
        for dy in (-1, 0, 1):
            for dx in (-1, 0, 1):
                d5y = 2 * dy + 2 - py + qy
                d5x = 2 * dx + 2 - px + qx
                if 0 <= d5y < 5 and 0 <= d5x < 5:
                    G[cc, dy + 1, dx + 1] = K5[d5y, d5x]
    return G


def _ph3_bands(chw, cvw, cqw):
    """-> [3, 36, 96]: band_dx[(cc*18+rr), (yo*6+o)] = G_o[cc, rr-yo, dx]."""
    specs = [(chw, 0, 0), (chw, 1, 1), (cvw, 0, 0), (cvw, 1, 1),
             (cqw, 1, 0), (cqw, 0, 1)]
    out = np.zeros((3, 36, 96), np.float32)
    for o, (K5, py, px) in enumerate(specs):
        G = _g_stencil(np.asarray(K5, np.float32)[0, 0], py, px)
        for dx in range(3):
            for yo in range(16):
                for dy in (-1, 0, 1):
                    rr = yo + dy + 1
                    if 0 <= rr < 18:
                        for cc in range(2):
                            out[dx, cc * 18 + rr, yo * 6 + o] = G[cc, dy + 1, dx]
    return out


def _selAB():
    """[96, 64]: cols 0..31 selA (g0n/g1n planes from m), 32..63 selB (den
    from E). Column j = plane*8 + row within a 32-col group (rows 24..31 of
    each group are zero pad so the PSUM block is fully written)."""
    sel = np.zeros((96, 64), np.float32)
    for yl in range(8):
        for c in range(WIDTH):
            p = yl * WIDTH + c
            pl = 0 if c < 6 else 1
            sel[p, pl * 8 + yl] = 1.0
            sel[p, 32 + 16 + yl] = 1.0
    return sel


# f32r weight tile column map: 18 banded blocks of 96 + ph3 3*96
WBCOL = {}
_c = 0
for _n in ("L1f0", "L1f1", "L1f2", "L1w0", "L1w1", "L1w2",
           "L2f0", "L2f1", "L2f2", "L2w0", "L2w1", "L2w2",
           "L3f0", "L3f1", "L3f2", "L3w0", "L3w1", "L3w2"):
    WBCOL[_n] = _c
    _c += 96
WBCOL["ph3"] = _c
_c += 288
WBCOLS = _c


def _host_prep(inputs):
    mosaic = np.asarray(inputs["mosaic"], np.float32)  # [B,4,512,512]
    B = mosaic.shape[0]
    mospad = np.zeros((B, 4, 522, 514), np.float32)
    mospad[:, :, 8:520, 1:513] = mosaic

    wb = np.zeros((120, WBCOLS), np.float32)
    for st, (k1, k2, k3) in (("f", ("fw0", "fw1", "fw2")),
                             ("w", ("ww0", "ww1", "ww2"))):
        b1 = _band(inputs[k1], 4, 10)    # [3, 40, 96]
        b2 = _band(inputs[k2], 12, 10)   # [3, 120, 96]
        b3 = _band(inputs[k3], 12, 10)
        for dx in range(3):
            wb[0:40, WBCOL[f"L1{st}{dx}"]:WBCOL[f"L1{st}{dx}"] + 96] = b1[dx]
            wb[0:120, WBCOL[f"L2{st}{dx}"]:WBCOL[f"L2{st}{dx}"] + 96] = b2[dx]
            wb[0:120, WBCOL[f"L3{st}{dx}"]:WBCOL[f"L3{st}{dx}"] + 96] = b3[dx]
    ph3 = _ph3_bands(inputs["chw"], inputs["cvw"], inputs["cqw"])
    for dx in range(3):
        wb[0:36, WBCOL["ph3"] + dx * 96:WBCOL["ph3"] + (dx + 1) * 96] = ph3[dx]
    return mospad, {"wr": wb, "ws": _selAB().astype(BF)}


# ---------------------------------------------------------------- kernel IR

def build_kernel(tc, outs, ins, ctx):
    nc = tc.nc
    mospad, wr, ws = ins["mospad"], ins["wr"], ins["ws"]
    out = outs["out"]

    wp = ctx.enter_context(tc.tile_pool(name="weights", bufs=1))
    pp = ctx.enter_context(tc.tile_pool(name="planes", bufs=1))
    rp = ctx.enter_context(tc.tile_pool(name="rings", bufs=1))
    asmp = ctx.enter_context(tc.tile_pool(name="asm", bufs=3))
    psc = ctx.enter_context(tc.tile_pool(name="psc", bufs=2, space="PSUM"))
    ps3 = ctx.enter_context(tc.tile_pool(name="ps3", bufs=2, space="PSUM"))
    pss = ctx.enter_context(tc.tile_pool(name="pss", bufs=2, space="PSUM"))

    wr_t = wp.tile([120, WBCOLS], F32R, tag="wr")
    nc.sync.dma_start(wr_t[:], wr)
    ws_t = wp.tile([96, 64], BF16, tag="ws")
    nc.sync.dma_start(ws_t[:], ws)

    def W(name, K):
        c = WBCOL[name]
        return wr_t[0:K, c:c + 96]

    # --- persistent planes
    mos0 = pp.tile([128, 2048], F32, tag="mos0")
    mos3 = pp.tile([128, 2048], F32, tag="mos3")
    m12 = pp.tile([128, 4096], F32, tag="m12")        # [c][t][512]
    g3 = pp.tile([128, 3 * 2048], F32, tag="g3")      # g0n | g1n | den
    c12 = pp.tile([128, 2 * 4 * 514], F32R, tag="c12")  # [c][t][514] padded
    qp6 = [pp.tile([128, 6 * 1024], F32, tag=f"qp6_{h}", name=f"qp6_{h}")
           for h in range(2)]
    zb = pp.tile([96, 1028], F32R, tag="zb")
    nc.gpsimd.memset(zb[:].bitcast(F32), 0.0)

    def load_plane(dst2048, ch):
        src = mospad[ch, 8:520, 1:513].rearrange(
            "(t p) x -> p t x", t=4).bitcast(F32)
        return nc.sync.dma_start(
            dst2048.rearrange("p (t x) -> p t x", t=4), src)

    pl_d = [load_plane(mos0[:], 0), load_plane(m12[:, 0:2048], 1),
            load_plane(m12[:, 2048:4096], 2), load_plane(mos3[:], 3)]

    nc.gpsimd.memset(
        c12[:].rearrange("p (c t x) -> p c t x", c=2, t=4)[
            :, :, :, 0:514:513].bitcast(F32), 0.0)

    # --- ring tiles (bf16). b1/b2: [120, 2*514] f|w combined, halo rows
    # 96..119; b0: [40, 514]; b3: [36, 514].
    b0_r = [rp.tile([40, 514], F32R, tag=f"b0_{i}", name=f"b0_{i}")
            for i in range(4)]
    b1_r = [rp.tile([120, 1028], F32R, tag=f"b1_{i}", name=f"b1_{i}")
            for i in range(4)]
    b2_r = [rp.tile([120, 1028], F32R, tag=f"b2_{i}", name=f"b2_{i}")
            for i in range(4)]
    b3_r = [rp.tile([36, 514], F32R, tag=f"b3_{i}", name=f"b3_{i}")
            for i in range(3)]
    b3s_r = [rp.tile([96, 512], F32, tag=f"b3s_{i}", name=f"b3s_{i}")
             for i in range(2)]
    m_r = [rp.tile([96, 512], BF16, tag=f"m_{i}", name=f"m_{i}")
           for i in range(2)]
    et_r = [rp.tile([96, 512], BF16, tag=f"et_{i}", name=f"et_{i}")
            for i in range(2)]
    for t in b1_r + b2_r:
        nc.gpsimd.memset(
            t[:].rearrange("p (h x) -> p h x", h=2)[:, :, 0:514:513].bitcast(
                F32), 0.0)
    for t in b3_r:
        nc.gpsimd.memset(t[:].bitcast(F32), 0.0)

    # --- helpers
    def conv3(psum_half, tile_, K, names, deps=()):
        mms = [nc.tensor.matmul(psum_half, W(names[dx], K),
                                tile_[0:K, dx:dx + 512],
                                start=(dx == 0), stop=(dx == 2))
               for dx in range(3)]
        for mm in mms:
            for d in deps:
                add_dep(mm, d, reason="rhs-ready")
        return mms

    def evict_fw(eng, dst, psum, s, k):
        """One relu-eviction for both stack halves: psum [96,1024] ->
        dst[0:96, (h,1:513)]; zero out-of-image rows on edge strips."""
        dv = dst[0:96, :].rearrange("p (h x) -> p h x", h=2)[:, :, 1:513]
        sv = psum[:].rearrange("p (h x) -> p h x", h=2)
        if eng is nc.scalar:
            a = eng.activation(dv, sv, RELU)
        else:
            a = eng.tensor_scalar_max(dv, sv, 0.0)
        if s == -1 and 8 - k > 0:
            z = nc.sync.dma_start(dst[0:(8 - k) * 12, :], zb[0:(8 - k) * 12, :])
            add_dep(z, a, reason="zero-top")
        if s == 63 and k > 0:
            z = nc.sync.dma_start(dst[(8 - k) * 12:96, :], zb[0:k * 12, :])
            add_dep(z, a, reason="zero-bot")
        return a

    def load_b0(s):
        t = b0_r[(s + 1) % 4]
        src = mospad[:, 8 * s + 8:8 * s + 18, :].transpose([1, 0, 2])
        d = nc.sync.dma_start(t[:], src)
        return t, [d]

    selpsum = [None, None]
    stg_cp = [[None, None], [None, None]]
    stg_r = [rp.tile([128, 512], F32, tag=f"stg_{i}", name=f"stg_{i}")
             for i in range(2)]

    def scatter_group(q):
        """Evict sel psum group q to staging, DMA -> g3 planes (rows
        32q-5..32q+26 clipped)."""
        ps = stg_r[q % 2]
        cps = [c for c in stg_cp[q % 2] if c is not None]
        ys0 = 32 * q - 5
        for pl in range(3):
            ya = max(ys0, 0)
            yend = min(ys0 + 32, HW)
            while ya < yend:
                g = (ya - ys0) // 8
                r0 = (ya - ys0) % 8
                run = min(8 - r0, yend - ya, 128 - (ya % 128))
                src = ps[32 * g + 8 * pl + r0:32 * g + 8 * pl + r0 + run, :]
                dst = g3[ya % 128:ya % 128 + run,
                         pl * 2048 + (ya // 128) * 512:
                         pl * 2048 + (ya // 128) * 512 + 512]
                d = nc.gpsimd.dma_start(dst, src)
                for c in cps:
                    add_dep(d, c, reason="stg-ready")
                ya += run

    def phase2(s, psfw):
        q, slot = (s + 1) // 4, (s + 1) % 4
        et = et_r[s % 2]
        mt = m_r[s % 2]
        psf = psfw[:].rearrange("p (h x) -> p h x", h=2)[:, 0, :]
        psw = psfw[:].rearrange("p (h x) -> p h x", h=2)[:, 1, :]
        nc.scalar.activation(et[:], psw, EXP)
        nc.vector.tensor_scalar_max(et[:], et[:], 1.0)
        nc.scalar.activation(mt[:], psf, RELU)
        nc.vector.tensor_mul(mt[:], mt[:], et[:])
        if slot % 2 == 0:
            selpsum[slot // 2] = pss.tile([64, 512], F32, tag="sel",
                                          name=f"sel{q}_{slot // 2}")
        ps = selpsum[slot // 2]
        so = 32 * (slot % 2)
        nc.tensor.matmul(ps[so:so + 32, :], ws_t[:, 0:32],
                         mt[:], start=True, stop=False,
                         tile_position=(0, so))
        nc.tensor.matmul(ps[so:so + 32, :], ws_t[:, 32:64],
                         et[:], start=False, stop=True,
                         tile_position=(0, so))
        if slot % 2 == 1 or s == 63:
            cp = nc.vector.tensor_copy(
                stg_r[q % 2][64 * (slot // 2):64 * (slot // 2) + 64, :],
                ps[:]) if slot % 2 == 1 else nc.vector.tensor_copy(
                stg_r[q % 2][0:32, :], ps[0:32, :])
            stg_cp[q % 2][slot // 2] = cp

    def phase25(t):
        den = g3[:, 4096 + t * 512:4096 + (t + 1) * 512]
        nc.vector.reciprocal(den, den)
        g01 = g3[:, 0:4096].rearrange("p (s c x) -> p s c x", s=2, c=4)[
            :, :, t, :]
        nc.vector.tensor_mul(g01, g01, den.unsqueeze(1).broadcast_to(
            [128, 2, 512]))
        m12v = m12[:].rearrange("p (c t x) -> p c t x", c=2, t=4)[:, :, t, :]
        c12v = c12[:].rearrange("p (c t x) -> p c t x", c=2, t=4)[
            :, :, t, 1:513]
        nc.vector.tensor_sub(c12v, m12v, g01)

    def phase3_strip(sq):
        t = b3_r[sq % 3]
        y0 = 16 * sq - 1
        dmas = []
        if y0 + 17 >= HW:  # sq == 31: rows 17, 35 must be zero
            dmas.append(nc.sync.dma_start(t[17:18, :], zb[0:1, 0:514]))
            dmas.append(nc.sync.dma_start(t[35:36, :], zb[0:1, 0:514]))
        ya, yb = max(y0, 0), min(y0 + 18, HW)
        while ya < yb:
            run = min(yb - ya, 128 - (ya % 128))
            tk = ya // 128
            for cc in range(2):
                off = cc * 2056 + tk * 514
                dmas.append(nc.gpsimd.dma_start(
                    t[cc * 18 + ya - y0:cc * 18 + ya - y0 + run, :],
                    c12[ya % 128:ya % 128 + run, off:off + 514]))
            ya += run
        p3 = ps3.tile([96, 512], F32, tag="p3", name=f"p3_{sq}")
        mms = [nc.tensor.matmul(p3[:, 0:512],
                                wr_t[0:36, WBCOL["ph3"] + dx * 96:
                                     WBCOL["ph3"] + dx * 96 + 96],
                                t[0:36, dx:dx + 512],
                                start=(dx == 0), stop=(dx == 2))
               for dx in range(3)]
        for mm in mms:
            for d in dmas:
                add_dep(mm, d, reason="b3-ready")
        s3 = b3s_r[sq % 2]
        if sq % 2 == 0:
            nc.scalar.copy(s3[:], p3[:, 0:512])
        else:
            nc.vector.tensor_copy(s3[:], p3[:, 0:512])
        yq = 16 * sq
        h, tlc = yq // 256, (yq // 128) % 2
        dst = qp6[h][yq % 128:yq % 128 + 16, :].rearrange(
            "p (o c x) -> p o c x", o=6, c=2)[:, :, tlc, :]
        nc.scalar.dma_start(dst, s3[:])

    asm_specs = [
        (0, 0, 0, 0, "mos0"), (0, 0, 1, None, "m1"),
        (0, 1, 0, 4, "g1"), (0, 1, 1, 3, "mos3"),
        (1, 0, 0, None, "mos0"), (1, 0, 1, None, "g0"),
        (1, 1, 0, None, "g1"), (1, 1, 1, None, "mos3"),
        (2, 0, 0, 2, "mos0"), (2, 0, 1, 5, "g0"),
        (2, 1, 0, None, "m2"), (2, 1, 1, 1, "mos3"),
    ]

    def plane_view(nm, t):
        if nm == "mos0":
            return mos0[:, t * 512:(t + 1) * 512]
        if nm == "mos3":
            return mos3[:, t * 512:(t + 1) * 512]
        if nm == "m1":
            return m12[:, t * 512:t * 512 + 512]
        if nm == "m2":
            return m12[:, 2048 + t * 512:2048 + t * 512 + 512]
        if nm == "g0":
            return g3[:, t * 512:t * 512 + 512]
        if nm == "g1":
            return g3[:, 2048 + t * 512:2048 + t * 512 + 512]

    def assemble_unit(arg):
        t, u = arg
        h, tl = t // 2, t % 2
        ch, py = u // 2, u % 2
        a = asmp.tile([128, 1024], F32, tag="asm", name="asm")
        prev = None
        ncopy = 0
        for (c_, py_, px, qo, nm) in asm_specs:
            if c_ != ch or py_ != py:
                continue
            view = a[:].rearrange("p (x two) -> p two x", two=2)[:, px, :]
            src = plane_view(nm, t)
            if qo is None:
                eng = nc.scalar if (u + ncopy) % 2 == 0 else nc.vector
                w_ = (eng.copy(view, src) if eng is nc.scalar
                      else eng.tensor_copy(view, src))
                ncopy += 1
            else:
                w_ = nc.vector.tensor_add(
                    view, qp6[h][:, qo * 1024 + tl * 512:
                                 qo * 1024 + tl * 512 + 512], src)
            if prev is not None:
                add_dep(w_, prev, reason="asm-interleave")
            prev = w_
        dst = out[ch].rearrange("(y two) x -> two y x", two=2)[
            py, t * 128:(t + 1) * 128, :]
        nc.scalar.dma_start(dst, a[:])

    # ---- main wavefront loop
    from collections import deque
    pending = deque()
    b1 = {}
    b2 = {}

    for i in range(69):
        s = i - 1
        if s <= 63:
            t0, d0 = load_b0(s)
            psum = psc.tile([96, 1024], F32, tag="fw", name=f"L1_{s}")
            ph = psum[:].rearrange("p (h x) -> p h x", h=2)
            for hi, st in ((0, "f"), (1, "w")):
                conv3(ph[:, hi, :], t0, 40,
                      [f"L1{st}{dx}" for dx in range(3)], deps=d0)
            b1[s] = b1_r[(s + 1) % 4]
            evict_fw(nc.scalar, b1[s], psum, s, 1)
        t2 = s - 2
        if -1 <= t2 <= 63:
            # halo rows for b1[t2]: one bf16 copy from next strip's tile
            if t2 < 63:
                hc = nc.vector.tensor_copy(b1[t2][96:120, :],
                                           b1[t2 + 1][0:24, :])
            else:
                hc = nc.sync.dma_start(b1[63][96:120, :], zb[0:24, :])
            psum = psc.tile([96, 1024], F32, tag="fw", name=f"L2_{t2}")
            ph = psum[:].rearrange("p (h x) -> p h x", h=2)
            for hi, st in ((0, "f"), (1, "w")):
                tv = b1[t2][:].rearrange("p (h x) -> p h x", h=2)[:, hi, :]
                mms = conv3(ph[:, hi, :], tv, 120,
                            [f"L2{st}{dx}" for dx in range(3)])
                for mm in mms:
                    add_dep(mm, hc, reason="halo-ready")
            b2[t2] = b2_r[(t2 + 1) % 4]
            evict_fw(nc.vector, b2[t2], psum, t2, 2)
        t3 = s - 4
        if -1 <= t3 <= 63:
            if t3 < 63:
                hc = nc.vector.tensor_copy(b2[t3][96:120, :],
                                           b2[t3 + 1][0:24, :])
            else:
                hc = nc.sync.dma_start(b2[63][96:120, :], zb[0:24, :])
            psfw = psc.tile([96, 1024], F32, tag="fw", name=f"L3_{t3}")
            ph = psfw[:].rearrange("p (h x) -> p h x", h=2)
            for hi, st in ((0, "f"), (1, "w")):
                tv = b2[t3][:].rearrange("p (h x) -> p h x", h=2)[:, hi, :]
                mms = conv3(ph[:, hi, :], tv, 120,
                            [f"L3{st}{dx}" for dx in range(3)])
                for mm in mms:
                    add_dep(mm, hc, reason="halo-ready")
            phase2(t3, psfw)
            if (t3 + 1) % 4 == 3:
                scatter_group((t3 + 1) // 4)
            if t3 == 63:
                scatter_group(16)
            b1.pop(t3, None)
            b2.pop(t3 - 1, None)
        # scatter(q) runs at s = 4q+6; phase25(t) needs groups <= 4t+4,
        # i.e. s >= 16t+22. Queue the chunk work and drain it smoothly
        # (2 units per iteration) to avoid burst stalls.
        if s >= 22 and (s - 22) % 16 == 0 and (s - 22) // 16 <= 2:
            tch = (s - 22) // 16
            phase25(tch)
            for sq in range(max(0, 8 * tch - 1), 8 * tch + 7):
                pending.append((phase3_strip, sq))
            if tch == 1:
                for u in range(6):
                    pending.append((assemble_unit, (0, u)))
            if tch == 2:
                for u in range(6):
                    pending.append((assemble_unit, (1, u)))
        for _ in range(2):
            if pending:
                fn, arg = pending.popleft()
                fn(arg)
    # tail
    while pending:
        fn, arg = pending.popleft()
        fn(arg)
    phase25(3)
    for sq in range(23, 32):
        phase3_strip(sq)
    for t in (2, 3):
        for u in range(6):
            assemble_unit((t, u))


# ---------------------------------------------------------------- execution

_CACHE = {}


def _get_compiled():
    if "nc" in _CACHE:
        return _CACHE["nc"]
    nc = bacc.Bacc("TRN2", target_bir_lowering=False, debug=False,
                   enable_asserts=False)
    ins = {
        "mospad": nc.dram_tensor("mospad", [4, 522, 514], F32R,
                                 kind="ExternalInput").ap(),
        "wr": nc.dram_tensor("wr", [120, WBCOLS], F32R,
                             kind="ExternalInput").ap(),
        "ws": nc.dram_tensor("ws", [96, 64], BF16,
                             kind="ExternalInput").ap(),
    }
    outs = {"out": nc.dram_tensor("out", [3, 1024, 1024], F32,
                                  kind="ExternalOutput").ap()}
    from contextlib import ExitStack
    with tile.TileContext(nc) as tc, ExitStack() as ctx:
        build_kernel(tc, outs, ins, ctx)
    nc.compile()
    _CACHE["nc"] = nc
    return nc


def kernel(**inputs):
    nc = _get_compiled()
    mospad, shared = _host_prep(inputs)
    in_maps = []
    for b in range(mospad.shape[0]):
        m = {"mospad": np.ascontiguousarray(mospad[b])}
        m.update(shared)
        in_maps.append(m)
    res = run_bass_kernel_spmd(nc, in_maps, core_ids=list(range(8)))
    return np.stack([res.results[b]["out"] for b in range(8)])
